# revision 1
# baseline (speedup 1.0000x reference)
"""GQA attention (dense_transformer) on 8 TRN2 NeuronCores.

Sharding: core c = b*4 + j  (b = batch 0..1, j = tensor-parallel rank 0..3).
Each core computes q-heads 8j..8j+7 (kv heads 2j, 2j+1) for batch b, then an
AllGather of attn^T over the 4 ranks of its batch group, then its 512-column
shard of the output projection.  Host assembles the full output.

Structure: projection t-chunks of 256 (SBUF residency), attention/AllGather/
wo windows of 512 (amortizes per-op overheads).  Causal diagonal tiles are
subranged (only the valid tq range is computed) plus one 128x128 triangle
mask.  All big matmuls run in float32r (full-rate PE, ~1e-4 rel precision);
the AllGather + wo tail runs in bf16.

Self-contained: hardcodes shapes from the problem spec.
"""
import os
import sys

sys.path.insert(0, "/opt/trn_rl_repo")

from collections import deque
from contextlib import ExitStack

import numpy as np
import ml_dtypes

import concourse.bass as bass
import concourse.mybir as mybir
import concourse.tile as tile
from concourse import bacc
from concourse.bass_utils import run_bass_kernel_spmd
from concourse.masks import make_identity

HIDDEN = 2048
N_HEADS = 32
N_KV_HEADS = 8
HEAD_DIM = 64
B_FULL, T_FULL = 2, 2048

NCORES = 8
NTP = 4                       # tensor-parallel ranks per batch group
NHL = N_HEADS // NTP          # 8 local q heads
NKVL = N_KV_HEADS // NTP      # 2 local kv heads
QF = NHL * HEAD_DIM           # 512 local q features
KF = NKVL * HEAD_DIM          # 128 local kv features
COLS = HIDDEN // NTP          # 512 output columns per rank
TCP = 256                     # projection t-chunk width
TCA = 512                     # attention window width
P = 128

F32 = mybir.dt.float32
F32R = mybir.dt.float32r
BF16 = mybir.dt.bfloat16

SCALE = 1.0 / np.sqrt(HEAD_DIM)

LAST_EXEC_NS = None
LAST_RESULTS = None


def build_kernel(T=T_FULL, repeat=1, no_ag=False, ag_mode='full8',
                 shared_cc=False, gsz=None, attn='v1', rep_mode='mm',
                 wo_mode='zeros', interleave=False, agq='sp', fine=False,
                 xbf=False, dmab=False):
    """One SPMD program; every core runs the same code on its shard."""
    assert T % TCA == 0
    NW = T // TCA             # attention windows
    KH = HIDDEN // P          # 16 k-tiles over hidden
    NTT = T // P              # tk tiles total
    WTK = TCA // P            # tk tiles per window (4)

    nc = bacc.Bacc("TRN2", debug=False)

    XDT = BF16 if xbf else F32R
    NCH = T // TCP
    if dmab:
        # chunk-major host layout: row p holds [c][k][t'] contiguous, so one
        # DMA per 256-col chunk (1 descriptor per partition)
        xT = nc.dram_tensor("xT", [P, NCH * KH * TCP], XDT,
                            kind="ExternalInput")
    else:
        xT = nc.dram_tensor("xT", [HIDDEN, T], XDT, kind="ExternalInput")
    wqT = nc.dram_tensor("wqT", [HIDDEN, QF], XDT, kind="ExternalInput")
    wkT = nc.dram_tensor("wkT", [HIDDEN, KF], XDT, kind="ExternalInput")
    wvT = nc.dram_tensor("wvT", [HIDDEN, KF], XDT, kind="ExternalInput")
    WOC = COLS // 2 if wo_mode in ('split', 'pps') else COLS
    WOR = HIDDEN if wo_mode in ('split', 'pps') else 2 * HIDDEN
    woT = nc.dram_tensor("woT", [WOR, WOC], BF16, kind="ExternalInput")
    cosT = nc.dram_tensor("cosT", [P, T], F32R, kind="ExternalInput")
    sinTs = nc.dram_tensor("sinTs", [P, T], F32R, kind="ExternalInput")
    swp = nc.dram_tensor("swp", [P, P], F32R, kind="ExternalInput")
    msk = nc.dram_tensor("msk", [P, P], F32R, kind="ExternalInput")
    out = nc.dram_tensor("out", [COLS, T], F32, kind="ExternalOutput")

    n_gather = NCORES if ag_mode in ('full8', 'single8', 'pp') else NTP
    cc_space = "Shared" if shared_cc else "Local"
    if ag_mode == 'pp':
        assert wo_mode == 'pps'
        GSZ = 1
        NPP = 4 * NW
        NCC = min(NPP * repeat, 2 * NPP)
        cc_in = [nc.dram_tensor(f"cc_in{i}", [P, TCA], BF16)
                 for i in range(NCC)]
        cc_out = [nc.dram_tensor(f"cc_out{i}", [NCORES * P, TCA], BF16,
                                 addr_space=cc_space)
                  for i in range(NCC)]
    elif ag_mode == 'ppd':
        # direct DMA into a shared gather buffer + tiny barrier collective
        assert wo_mode == 'pps' and shared_cc
        GSZ = 1
        NGB = min(2 * NW, NW * repeat) if repeat > 1 else NW
        gat = [nc.dram_tensor(f"gat{i}", [4 * NCORES * P, TCA], BF16,
                              addr_space="Shared") for i in range(NGB)]
        bar_in = [nc.dram_tensor(f"barin{i}", [1, 1], BF16)
                  for i in range(NGB)]
        bar_out = [nc.dram_tensor(f"barout{i}", [NCORES, 1], BF16)
                   for i in range(NGB)]
        cc_in = cc_out = []
    elif ag_mode == 'single8':
        cc_in = [nc.dram_tensor(f"cc_in{i}", [QF, T], BF16)
                 for i in range(repeat)]
        cc_out = [nc.dram_tensor(f"cc_out{i}", [n_gather * QF, T], BF16,
                                 addr_space=cc_space)
                  for i in range(repeat)]
    else:
        if gsz is None:
            GSZ = 2 if NW % 2 == 0 else 1  # windows gathered per collective
        else:
            GSZ = gsz
        assert NW % GSZ == 0
        NPAIR = NW // GSZ
        NCC = min(NPAIR * repeat, 2 * NPAIR)
        cc_in = [nc.dram_tensor(f"cc_in{i}", [QF, GSZ * TCA], BF16)
                 for i in range(NCC)]
        cc_out = [nc.dram_tensor(f"cc_out{i}", [n_gather * QF, GSZ * TCA], BF16,
                                 addr_space=cc_space)
                  for i in range(NCC)]
    groups = ([[0, 1, 2, 3, 4, 5, 6, 7]] if ag_mode in ('full8', 'pp', 'ppd')
              else [[0, 1, 2, 3], [4, 5, 6, 7]])

    v2 = (attn == 'v2')
    VDT = BF16 if v2 else F32R      # vaug / p dtype
    # engine whose DGE queue carries the gathered-tile loads; 'pool' keeps
    # their wait-on-collective off the main SP DMA queue
    

    with tile.TileContext(nc) as tc, ExitStack() as est:
        consts = est.enter_context(tc.tile_pool(name="consts", bufs=1))
        kpool = est.enter_context(tc.tile_pool(name="kpool", bufs=1))
        xcpool = est.enter_context(
            tc.tile_pool(name="xcpool", bufs=2 if dmab else 17))
        stream = est.enter_context(tc.tile_pool(name="stream", bufs=3))
        qrpool = est.enter_context(tc.tile_pool(name="qrpool", bufs=6))
        ppool = est.enter_context(tc.tile_pool(name="ppool", bufs=8 if v2 else 4))
        atpool = est.enter_context(tc.tile_pool(name="atpool", bufs=6))
        agpool = est.enter_context(
            tc.tile_pool(name="agpool", bufs=4 if dmab else 32))
        small = est.enter_context(tc.tile_pool(name="small", bufs=2))
        reppool = (est.enter_context(tc.tile_pool(name="reppool", bufs=2))
                   if v2 else None)
        ps_proj = est.enter_context(tc.tile_pool(name="ps_proj",
                                                 bufs=1 if v2 else 2, space="PSUM"))
        ps_s = est.enter_context(tc.tile_pool(name="ps_s",
                                              bufs=4 if v2 else 2, space="PSUM"))
        ps_pv = est.enter_context(tc.tile_pool(name="ps_pv", bufs=2, space="PSUM"))
        ps_y = est.enter_context(tc.tile_pool(name="ps_y", bufs=1, space="PSUM"))
        if v2:
            ps_misc = ps_proj         # rope swap / V transpose share proj bank
            MTAG = "proj"
        else:
            ps_misc = est.enter_context(
                tc.tile_pool(name="ps_misc", bufs=1, space="PSUM"))
            MTAG = "misc"

        # ---- constants (DMA order matters for startup: weights first, then
        # rope tables, mask, wo) ----
        swp_sb = consts.tile([P, P], F32R)
        wq_sb = consts.tile([P, KH, QF], XDT)
        wk_sb = consts.tile([P, KH, KF], XDT)
        wv_sb = consts.tile([P, KH, KF], XDT)
        wo_sb = consts.tile([P, WOR // P, WOC], BF16)
        cos_sb = consts.tile([P, 2, TCA], F32R)
        sin_sb = consts.tile([P, 2, TCA], F32R)
        msk_sb = consts.tile([P, P], F32R)
        id_sb = consts.tile([P, P], F32R)
        id_f32 = consts.tile([P, P], F32)
        ones_sb = consts.tile([1, HEAD_DIM], F32R)
        ones_f32 = consts.tile([P, 1], F32)
        ones_row_f32 = consts.tile([1, HEAD_DIM], F32)

        if dmab:
            xv2 = xT[:, :].rearrange("p (c k t) -> p c k t", c=NCH, k=KH)
            xv = None
        else:
            xv = xT[:, :].rearrange("(t p) n -> p t n", p=P)
        nc.sync.dma_start(out=swp_sb, in_=swp[:, :])
        wqv = wqT[:, :].rearrange("(t p) f -> p t f", p=P)
        wkv = wkT[:, :].rearrange("(t p) f -> p t f", p=P)
        wvv = wvT[:, :].rearrange("(t p) f -> p t f", p=P)
        wov = woT[:, :].rearrange("(t p) f -> p t f", p=P)
        xc0 = []
        if dmab:
            for k in range(KH):
                nc.sync.dma_start(out=wq_sb[:, k, :], in_=wqv[:, k, :])
            xt0 = xcpool.tile([P, KH, TCP], XDT, tag="xc")
            nc.sync.dma_start(out=xt0, in_=xv2[:, 0, :, :])
            xc0 = [xt0[:, k, :] for k in range(KH)]
        else:
            for k in range(KH):
                nc.sync.dma_start(out=wq_sb[:, k, :], in_=wqv[:, k, :])
                xt_ = xcpool.tile([P, TCP], XDT, tag="xc")
                nc.sync.dma_start(out=xt_, in_=xv[:, k, 0:TCP])
                xc0.append(xt_)
        for k in range(KH):
            nc.sync.dma_start(out=wk_sb[:, k, :], in_=wkv[:, k, :])
            nc.sync.dma_start(out=wv_sb[:, k, :], in_=wvv[:, k, :])
        sl = slice(0, TCA)
        nc.sync.dma_start(out=cos_sb[:, 0, :], in_=cosT[:, sl])
        nc.sync.dma_start(out=sin_sb[:, 0, :], in_=sinTs[:, sl])
        nc.sync.dma_start(out=msk_sb, in_=msk[:, :])

        make_identity(nc, id_f32)
        nc.vector.tensor_copy(id_sb, id_f32)
        nc.vector.memset(ones_f32, 1.0)
        nc.vector.memset(ones_row_f32, 1.0)
        nc.vector.tensor_copy(ones_sb, ones_row_f32)

        # ---- persistent K / V accumulators ----
        KA = kpool.tile([P, T], F32R, tag="KA")   # [g0; g0] roped K^T
        KB = kpool.tile([P, T], F32R, tag="KB")   # [g1; g1]
        # V natural layout per tk-tile: cols = [V_g0 (64) | 1 | V_g1 (64) | 1]
        vaug = kpool.tile([P, NTT, 2 * HEAD_DIM + 2], VDT, tag="vaug")
        if v2:
            msk_v = consts.tile([P, P], VDT)
            nc.vector.tensor_copy(msk_v, msk_sb)
        else:
            msk_v = msk_sb
        for t in range(NTT):
            nc.vector.tensor_copy(vaug[:, t, HEAD_DIM:HEAD_DIM + 1], ones_f32)
            nc.vector.tensor_copy(vaug[:, t, 2 * HEAD_DIM + 1:2 * HEAD_DIM + 2],
                                  ones_f32)


        def rope(raw_sb, cs, ss, out_ap):
            """out = raw*cos + swap(raw)*sin_signed  (all [P, TCP])."""
            sw_ps = ps_misc.tile([P, TCP], F32, tag=MTAG)
            nc.tensor.matmul(sw_ps, lhsT=swp_sb, rhs=raw_sb, start=True, stop=True)
            m2 = stream.tile([P, TCP], F32R, tag="tmp")
            nc.vector.tensor_tensor(out=m2, in0=sw_ps, in1=ss, op=mybir.AluOpType.mult)
            nc.vector.tensor_tensor(out=out_ap, in0=raw_sb, in1=cs,
                                    op=mybir.AluOpType.mult)
            nc.vector.tensor_tensor(out=out_ap, in0=out_ap, in1=m2,
                                    op=mybir.AluOpType.add)

        def project_pieces(c, qrope, half, xc=None):
            """Projections + rope for t-chunk c as a list of thunks."""
            csl = slice(c * TCP, (c + 1) * TCP)
            hsl = slice(half * TCP, (half + 1) * TCP)
            slot = (c // 2) % 2
            lsl_c = slice((c % 2) * TCP, (c % 2 + 1) * TCP)
            cs = cos_sb[:, slot, lsl_c]
            ss = sin_sb[:, slot, lsl_c]
            xcl = list(xc) if xc is not None else []
            pieces = []

            def xc_dma():
                if dmab:
                    xt_ = xcpool.tile([P, KH, TCP], XDT, tag="xc",
                                      name=f"xch{c}")
                    nc.sync.dma_start(out=xt_, in_=xv2[:, c, :, :])
                    xcl.extend(xt_[:, k, :] for k in range(KH))
                else:
                    for k in range(KH):
                        t_ = xcpool.tile([P, TCP], XDT, tag="xc",
                                         name=f"xc{c}_{k}")
                        nc.sync.dma_start(out=t_, in_=xv[:, k, csl])
                        xcl.append(t_)
            if xc is None:
                pieces.append(xc_dma)

            def qgroup(m):
                st = {}

                def fa():
                    st['q_ps'] = ps_proj.tile([P, TCP], F32, tag="proj",
                                              name=f"qps{c}_{m}")
                    for k in range(KH // 2):
                        nc.tensor.matmul(st['q_ps'],
                                         lhsT=wq_sb[:, k, m * P:(m + 1) * P],
                                         rhs=xcl[k], start=(k == 0),
                                         stop=False)

                def fb():
                    q_ps = st['q_ps']
                    for k in range(KH // 2, KH):
                        nc.tensor.matmul(q_ps,
                                         lhsT=wq_sb[:, k, m * P:(m + 1) * P],
                                         rhs=xcl[k], start=False,
                                         stop=(k == KH - 1))
                    raw = stream.tile([P, TCP], F32R, tag="raw",
                                      name=f"qraw{c}_{m}")
                    nc.vector.tensor_copy(raw, q_ps)
                    rope(raw, cs, ss, qrope[m][:, hsl])

                def f():
                    fa()
                    fb()
                return [fa, fb] if fine else [f]
            for m in range(4):
                pieces.extend(qgroup(m))

            def kgroup():
                k_ps = ps_proj.tile([P, TCP], F32, tag="proj")
                for k in range(KH):
                    nc.tensor.matmul(k_ps, lhsT=wk_sb[:, k, :], rhs=xcl[k],
                                     start=(k == 0), stop=(k == KH - 1))
                kraw = stream.tile([P, TCP], F32R, tag="raw")
                nc.vector.tensor_copy(kraw, k_ps)
                krope = stream.tile([P, TCP], F32R, tag="raw")
                rope(kraw, cs, ss, krope)
                nc.vector.tensor_copy(KA[0:64, csl], krope[0:64, :])
                nc.vector.tensor_copy(KA[64:128, csl], krope[0:64, :])
                nc.vector.tensor_copy(KB[0:64, csl], krope[64:128, :])
                nc.vector.tensor_copy(KB[64:128, csl], krope[64:128, :])
            pieces.append(kgroup)

            def vgroup():
                v_ps = ps_proj.tile([P, TCP], F32, tag="proj")
                for k in range(KH):
                    nc.tensor.matmul(v_ps, lhsT=wv_sb[:, k, :], rhs=xcl[k],
                                     start=(k == 0), stop=(k == KH - 1))
                vt = stream.tile([P, TCP], F32R, tag="raw")
                nc.vector.tensor_copy(vt, v_ps)
                for tt in range(TCP // P):
                    tp_ps = ps_misc.tile([P, P], F32R, tag=MTAG)
                    nc.tensor.transpose(tp_ps, vt[:, tt * P:(tt + 1) * P], id_sb)
                    tkt = c * (TCP // P) + tt
                    nc.vector.tensor_copy(vaug[:, tkt, 0:HEAD_DIM],
                                          tp_ps[:, 0:HEAD_DIM])
                    nc.vector.tensor_copy(
                        vaug[:, tkt, HEAD_DIM + 1:2 * HEAD_DIM + 1],
                        tp_ps[:, HEAD_DIM:2 * HEAD_DIM])
            pieces.append(vgroup)
            return pieces

        def project(c, qrope, half, xc=None):
            for pc in project_pieces(c, qrope, half, xc=xc):
                pc()

        NKW = 2 * KH if ag_mode == 'full8' else KH

        def emit_wo_ppd(w0, gi, bar):
            osl = slice(w0 * TCA, (w0 + 1) * TCA)
            gv = gat[gi][:, :].rearrange("(t p) n -> p t n", p=P)
            ag = []
            for pr in range(4):
                for r in range(NCORES):
                    t_ = agpool.tile([P, TCA], BF16, tag="ag",
                                     name=f"agd{w0}_{pr}_{r}")
                    d = nc.sync.dma_start(out=t_, in_=gv[:, pr * 8 + r, :])
                    if bar is not None:
                        bass._add_dep_helper(d.ins, bar.ins, sync=True,
                                             reason="ag waits ppd barrier")
                    ag.append(t_)
            for b_ in range(2):
                for m in range(2):
                    y_ps = ps_y.tile([P, TCA], F32, tag="y")
                    kk = 0
                    for pr in range(4):
                        for j in range(4):
                            nc.tensor.matmul(
                                y_ps,
                                lhsT=wo_sb[:, pr * 4 + j, m * P:(m + 1) * P],
                                rhs=ag[pr * 8 + b_ * 4 + j],
                                start=(kk == 0), stop=(kk == 15))
                            kk += 1
                    y_sb = small.tile([P, TCA], F32, tag="ysb")
                    nc.vector.tensor_copy(y_sb, y_ps)
                    r0 = (b_ * 2 + m) * P
                    nc.sync.dma_start(out=out[r0:r0 + P, osl], in_=y_sb)

        def emit_wo_pp(w0, i0):
            osl = slice(w0 * TCA, (w0 + 1) * TCA)
            ag = []
            eng = nc.gpsimd if agq == 'pool' else nc.sync
            for pr in range(4):
                ccv = cc_out[i0 + pr][:, :].rearrange("(t p) n -> p t n", p=P)
                for r in range(NCORES):
                    t_ = agpool.tile([P, TCA], BF16, tag="ag",
                                     name=f"ag{w0}_{pr}_{r}")
                    eng.dma_start(out=t_, in_=ccv[:, r, :])
                    ag.append(t_)
            for b_ in range(2):
                for m in range(2):
                    y_ps = ps_y.tile([P, TCA], F32, tag="y")
                    kk = 0
                    for pr in range(4):
                        for j in range(4):
                            nc.tensor.matmul(
                                y_ps,
                                lhsT=wo_sb[:, pr * 4 + j, m * P:(m + 1) * P],
                                rhs=ag[pr * 8 + b_ * 4 + j],
                                start=(kk == 0), stop=(kk == 15))
                            kk += 1
                    y_sb = small.tile([P, TCA], F32, tag="ysb")
                    nc.vector.tensor_copy(y_sb, y_ps)
                    r0 = (b_ * 2 + m) * P
                    nc.sync.dma_start(out=out[r0:r0 + P, osl], in_=y_sb)

        def emit_wo(pi, w0):
            ccv = cc_out[pi][:, :].rearrange("(t p) n -> p t n", p=P)
            for sw in range(GSZ):
                ssl = slice(sw * TCA, (sw + 1) * TCA)
                osl = slice((w0 + sw) * TCA, (w0 + sw + 1) * TCA)
                ag = []
                eng = nc.gpsimd if agq == 'pool' else nc.sync
                for k in range(NKW):
                    ag_t = agpool.tile([P, TCA], BF16, tag="ag")
                    eng.dma_start(out=ag_t, in_=ccv[:, k, ssl])
                    ag.append(ag_t)
                if wo_mode == 'split':
                    # 256 output cols for BOTH batches: row (2b+m)*128 of out.
                    for b_ in range(2):
                        for m in range(2):
                            y_ps = ps_y.tile([P, TCA], F32, tag="y")
                            for k in range(KH):
                                nc.tensor.matmul(
                                    y_ps, lhsT=wo_sb[:, k, m * P:(m + 1) * P],
                                    rhs=ag[b_ * KH + k], start=(k == 0),
                                    stop=(k == KH - 1))
                            y_sb = small.tile([P, TCA], F32, tag="ysb")
                            nc.vector.tensor_copy(y_sb, y_ps)
                            r0 = (b_ * 2 + m) * P
                            nc.sync.dma_start(out=out[r0:r0 + P, osl], in_=y_sb)
                else:
                    for m in range(4):
                        y_ps = ps_y.tile([P, TCA], F32, tag="y")
                        for k in range(NKW):
                            nc.tensor.matmul(y_ps,
                                             lhsT=wo_sb[:, k, m * P:(m + 1) * P],
                                             rhs=ag[k], start=(k == 0),
                                             stop=(k == NKW - 1))
                        y_sb = small.tile([P, TCA], F32, tag="ysb")
                        nc.vector.tensor_copy(y_sb, y_ps)
                        nc.sync.dma_start(out=out[m * P:(m + 1) * P, osl], in_=y_sb)

        def emit_wo_pieces(pi, w0):
            ccv = cc_out[pi][:, :].rearrange("(t p) n -> p t n", p=P)
            out_pieces = []
            for sw in range(GSZ):
                ssl = slice(sw * TCA, (sw + 1) * TCA)
                osl = slice((w0 + sw) * TCA, (w0 + sw + 1) * TCA)
                ag = []

                def ags(ssl=ssl, ag=ag):
                    eng = nc.gpsimd if agq == 'pool' else nc.sync
                    if dmab:
                        for k0 in range(0, NKW, 8):
                            ag_t = agpool.tile([P, 8, TCA], BF16, tag="ag")
                            eng.dma_start(out=ag_t,
                                          in_=ccv[:, k0:k0 + 8, ssl])
                            ag.extend(ag_t[:, kk, :] for kk in range(8))
                    else:
                        for k in range(NKW):
                            ag_t = agpool.tile([P, TCA], BF16, tag="ag")
                            eng.dma_start(out=ag_t, in_=ccv[:, k, ssl])
                            ag.append(ag_t)
                out_pieces.append(ags)
                if wo_mode in ('split', 'pps'):
                    def ygrp(b_, m, ag=ag, osl=osl):
                        def f():
                            y_ps = ps_y.tile([P, TCA], F32, tag="y")
                            for k in range(KH):
                                nc.tensor.matmul(
                                    y_ps, lhsT=wo_sb[:, k, m * P:(m + 1) * P],
                                    rhs=ag[b_ * KH + k], start=(k == 0),
                                    stop=(k == KH - 1))
                            y_sb = small.tile([P, TCA], F32, tag="ysb")
                            nc.vector.tensor_copy(y_sb, y_ps)
                            r0 = (b_ * 2 + m) * P
                            nc.sync.dma_start(out=out[r0:r0 + P, osl], in_=y_sb)
                        return f
                    out_pieces += [ygrp(b_, m)
                                   for b_ in range(2) for m in range(2)]
                else:
                    def ygrp(m, ag=ag, osl=osl):
                        def f():
                            y_ps = ps_y.tile([P, TCA], F32, tag="y")
                            for k in range(NKW):
                                nc.tensor.matmul(
                                    y_ps, lhsT=wo_sb[:, k, m * P:(m + 1) * P],
                                    rhs=ag[k], start=(k == 0),
                                    stop=(k == NKW - 1))
                            y_sb = small.tile([P, TCA], F32, tag="ysb")
                            nc.vector.tensor_copy(y_sb, y_ps)
                            nc.sync.dma_start(out=out[m * P:(m + 1) * P, osl],
                                              in_=y_sb)
                        return f
                    out_pieces += [ygrp(m) for m in range(4)]
            return out_pieces

        def emit_wo_pp_pieces(w0, i0):
            osl = slice(w0 * TCA, (w0 + 1) * TCA)
            ag = []

            def ags():
                eng = nc.gpsimd if agq == 'pool' else nc.sync
                for pr in range(4):
                    ccv = cc_out[(i0 + pr) % NCC][:, :].rearrange(
                        "(t p) n -> p t n", p=P)
                    for r in range(NCORES):
                        t_ = agpool.tile([P, TCA], BF16, tag="ag",
                                         name=f"agp{w0}_{pr}_{r}")
                        eng.dma_start(out=t_, in_=ccv[:, r, :])
                        ag.append(t_)

            def ygrp(b_, m):
                def f():
                    y_ps = ps_y.tile([P, TCA], F32, tag="y")
                    kk = 0
                    for pr in range(4):
                        for j in range(4):
                            nc.tensor.matmul(
                                y_ps,
                                lhsT=wo_sb[:, pr * 4 + j, m * P:(m + 1) * P],
                                rhs=ag[pr * 8 + b_ * 4 + j],
                                start=(kk == 0), stop=(kk == 15))
                            kk += 1
                    y_sb = small.tile([P, TCA], F32, tag="ysb")
                    nc.vector.tensor_copy(y_sb, y_ps)
                    r0 = (b_ * 2 + m) * P
                    nc.sync.dma_start(out=out[r0:r0 + P, osl], in_=y_sb)
                return f
            return [ags] + [ygrp(b_, m) for b_ in range(2) for m in range(2)]

        def attn_head(rep, w, h, n_tk, qrope, at_tiles):
            g = h // (NHL // NKVL)
            par = h % 2
            base = par * HEAD_DIM
            ksrc = KA if g == 0 else KB
            qt = qrope[h // 2]
            lsl = slice(base, base + HEAD_DIM)

            pv_ps = ps_pv.tile([HEAD_DIM + 1, TCA], F32, tag="pv")
            for i in range(n_tk):
                o = i - w * WTK
                lo = max(o, 0) * P
                s_ps = ps_s.tile([P, TCA], F32, tag="s")
                nc.tensor.matmul(
                    s_ps[:, lo:],
                    lhsT=ksrc[lsl, i * P:(i + 1) * P],
                    rhs=qt[lsl, lo:],
                    start=True, stop=True)
                p_sb = ppool.tile([P, TCA], F32R, tag="p")
                nc.scalar.activation(out=p_sb[:, lo:], in_=s_ps[:, lo:],
                                     func=mybir.ActivationFunctionType.Exp,
                                     scale=float(SCALE))
                if o >= 0:
                    nc.vector.tensor_tensor(out=p_sb[:, lo:lo + P],
                                            in0=p_sb[:, lo:lo + P],
                                            in1=msk_sb,
                                            op=mybir.AluOpType.mult)
                vsl = slice(g * (HEAD_DIM + 1), (g + 1) * (HEAD_DIM + 1))
                nc.tensor.matmul(pv_ps[:, lo:], lhsT=vaug[:, i, vsl],
                                 rhs=p_sb[:, lo:],
                                 start=(i == 0), stop=(i == n_tk - 1))

            rec = small.tile([1, TCA], F32R, tag="recip")
            with nc.allow_low_precision(reason="f32r softmax denom"):
                nc.vector.reciprocal(rec, pv_ps[HEAD_DIM:HEAD_DIM + 1, :])
            rep_ps = ps_misc.tile([HEAD_DIM, TCA], F32, tag="misc")
            nc.tensor.matmul(rep_ps, lhsT=ones_sb, rhs=rec,
                             start=True, stop=True)
            rep_sb = small.tile([HEAD_DIM, TCA], F32, tag="rep")
            nc.scalar.activation(out=rep_sb, in_=rep_ps,
                                 func=mybir.ActivationFunctionType.Copy)
            nc.vector.tensor_tensor(
                out=at_tiles[h // 2][base:base + HEAD_DIM, :],
                in0=pv_ps[0:HEAD_DIM, :], in1=rep_sb,
                op=mybir.AluOpType.mult)
            if ag_mode == 'pp' and h % 2 == 1:
                ppi = ((rep * NW + w) * 4 + h // 2) % NCC
                nc.sync.dma_start(out=cc_in[ppi][:, :],
                                  in_=at_tiles[h // 2])
                if not no_ag:
                    nc.gpsimd.collective_compute(
                        "AllGather", mybir.AluOpType.bypass,
                        replica_groups=groups,
                        ins=[cc_in[ppi][:, :]],
                        outs=[cc_out[ppi][:, :]],
                    )
            elif ag_mode == 'ppd' and h % 2 == 1:
                pr_ = h // 2
                gi_ = (rep * NW + w) % NGB
                ap0 = gat[gi_][0:P, :]
                apw = bass.AP(
                    ap0.tensor,
                    pid_ofs + pr_ * (NCORES * P * TCA),
                    ap0.ap,
                    dep_tracking_offset=pr_ * (NCORES * P * TCA))
                d = nc.sync.dma_start(out=apw,
                                      in_=at_tiles[pr_][:, :])
                win_dmas.append(d)

        if ag_mode == 'ppd':
            pid_ofs = nc.sync.partition_id() * (P * TCA)
        win_dmas = []
        pending_wo = None
        if interleave:
            assert not v2 and ag_mode in ('full8', 'pp')
            NG = NW * repeat
            qrope_cur = [qrpool.tile([P, TCA], F32R, tag="qrope",
                                     name=f"qr0_{m}") for m in range(4)]
            project(0, qrope_cur, 0, xc=xc0)
            project(1, qrope_cur, 1)
            pending = deque()
            for gw in range(NG):
                rep, w = divmod(gw, NW)
                n_tk = (w + 1) * WTK
                at_tiles = [atpool.tile([P, TCA], BF16, tag="attnT",
                                        name=f"at{gw}_{m}") for m in range(4)]
                pieces = deque()
                qrope_next = qrope_cur
                if gw + 1 < NG:
                    w1 = (w + 1) % NW
                    nsl = slice(w1 * TCA, (w1 + 1) * TCA)
                    nc.sync.dma_start(out=cos_sb[:, w1 % 2, :],
                                      in_=cosT[:, nsl])
                    nc.sync.dma_start(out=sin_sb[:, w1 % 2, :],
                                      in_=sinTs[:, nsl])
                    qrope_next = [qrpool.tile([P, TCA], F32R, tag="qrope",
                                              name=f"qr{gw + 1}_{m}")
                                  for m in range(4)]
                    pieces.extend(project_pieces(2 * w1, qrope_next, 0))
                    pieces.extend(project_pieces(2 * w1 + 1, qrope_next, 1))
                pi = (rep * (NW // GSZ) + w // GSZ) % NCC
                psl = slice((w % GSZ) * TCA, (w % GSZ + 1) * TCA)
                budget = 3 if fine else 2
                for h in range(NHL):
                    attn_head(rep, w, h, n_tk, qrope_cur, at_tiles)
                    if fine and ag_mode == 'full8' and h % 2 == 1:
                        m = h // 2
                        nc.sync.dma_start(
                            out=cc_in[pi][m * P:(m + 1) * P, psl],
                            in_=at_tiles[m])
                    if pending and h >= 2:
                        pending.popleft()()
                    for _ in range(budget):
                        if pieces:
                            pieces.popleft()()
                while pieces:
                    pieces.popleft()()
                while pending:
                    pending.popleft()()
                if ag_mode == 'pp':
                    pending = deque(
                        emit_wo_pp_pieces(w, ((rep * NW + w) * 4) % NCC))
                else:
                    if not fine:
                        for m in range(4):
                            nc.sync.dma_start(
                                out=cc_in[pi][m * P:(m + 1) * P, psl],
                                in_=at_tiles[m])
                    if w % GSZ == GSZ - 1:
                        if not no_ag:
                            nc.gpsimd.collective_compute(
                                "AllGather", mybir.AluOpType.bypass,
                                replica_groups=groups,
                                ins=[cc_in[pi][:, :]],
                                outs=[cc_out[pi][:, :]],
                            )
                        pending = deque(emit_wo_pieces(pi, w - GSZ + 1))
                if gw == 0:
                    for k in range(WOR // P):
                        nc.sync.dma_start(out=wo_sb[:, k, :], in_=wov[:, k, :])
                qrope_cur = qrope_next
            while pending:
                pending.popleft()()
            repeat_range = []
        else:
            repeat_range = range(repeat)
        for rep in repeat_range:
            for w in range(NW):
                wsl = slice(w * TCA, (w + 1) * TCA)
                qrope = []
                for _ in range(4):
                    qr_t = qrpool.tile([P, TCA], F32R, tag="qrope")
                    qrope.append(qr_t)
                first = (rep == 0 and w == 0)
                project(2 * w, qrope, 0, xc=xc0 if first else None)
                project(2 * w + 1, qrope, 1)

                # ---- attention window ----
                n_tk = (w + 1) * WTK
                at_tiles = []
                for _ in range(4):
                    at_t = atpool.tile([P, TCA], BF16, tag="attnT")
                    at_tiles.append(at_t)
                if not v2:
                    for h in range(NHL):
                        attn_head(rep, w, h, n_tk, qrope, at_tiles)
                        if h == 3 and pending_wo is not None:
                            pending_wo()
                            pending_wo = None
                else:
                    # Row-tiled attention: heads (2pr, 2pr+1) live in SBUF
                    # partition halves 0-63 / 64-127, so their score matmuls
                    # (contract=64) auto-place at PE row quadrants (0,0) and
                    # (64,0) and run concurrently.  Chunked score/PV phases
                    # limit PE mode switches.
                    CH = 2
                    for pr in range(4):
                        g = pr // 2
                        ksrc = KA if g == 0 else KB
                        qt = qrope[pr]
                        vsl = slice(g * (HEAD_DIM + 1), (g + 1) * (HEAD_DIM + 1))
                        pv = [ps_pv.tile([HEAD_DIM + 1, TCA], F32, tag="pv",
                                         name=f"pv{pr}_{hh}")
                              for hh in range(2)]
                        for c0 in range(0, n_tk, CH):
                            cur = []
                            for i in range(c0, min(c0 + CH, n_tk)):
                                o = i - w * WTK
                                lo = max(o, 0) * P
                                ssl = slice(i * P, (i + 1) * P)
                                sp = [ps_s.tile([P, TCA], F32, tag="s",
                                                name=f"s{i}_{k_}")
                                      for k_ in range(2)]
                                nc.tensor.matmul(sp[0][:, lo:],
                                                 lhsT=ksrc[0:HEAD_DIM, ssl],
                                                 rhs=qt[0:HEAD_DIM, lo:],
                                                 start=True, stop=True)
                                nc.tensor.matmul(sp[1][:, lo:],
                                                 lhsT=ksrc[HEAD_DIM:P, ssl],
                                                 rhs=qt[HEAD_DIM:P, lo:],
                                                 start=True, stop=True)
                                pp = [ppool.tile([P, TCA], VDT, tag="p",
                                                 name=f"p{i}_{k_}")
                                      for k_ in range(2)]
                                for s_, p_ in zip(sp, pp):
                                    nc.scalar.activation(
                                        out=p_[:, lo:], in_=s_[:, lo:],
                                        func=mybir.ActivationFunctionType.Exp,
                                        scale=float(SCALE))
                                    if o >= 0:
                                        nc.vector.tensor_tensor(
                                            out=p_[:, lo:lo + P],
                                            in0=p_[:, lo:lo + P], in1=msk_v,
                                            op=mybir.AluOpType.mult)
                                cur.append((i, lo, pp))
                            for (i, lo, pp) in cur:
                                for hh in range(2):
                                    nc.tensor.matmul(
                                        pv[hh][:, lo:], lhsT=vaug[:, i, vsl],
                                        rhs=pp[hh][:, lo:],
                                        start=(i == 0), stop=(i == n_tk - 1))
                        for hh in range(2):
                            base = hh * HEAD_DIM
                            rec = small.tile([1, TCA], F32R, tag="recip")
                            with nc.allow_low_precision(reason="f32r denom"):
                                nc.vector.reciprocal(
                                    rec, pv[hh][HEAD_DIM:HEAD_DIM + 1, :])
                            rep_sb = reppool.tile([HEAD_DIM, TCA], F32R,
                                                  tag="rep")
                            if rep_mode == 'mm':
                                rep_ps = ps_s.tile([HEAD_DIM, TCA], F32, tag="s",
                                                   name=f"rp{pr}_{hh}")
                                nc.tensor.matmul(rep_ps, lhsT=ones_sb, rhs=rec,
                                                 start=True, stop=True)
                                nc.scalar.activation(
                                    out=rep_sb, in_=rep_ps,
                                    func=mybir.ActivationFunctionType.Copy)
                            else:
                                nc.gpsimd.partition_broadcast(
                                    rep_sb, rec[0:1, :], channels=HEAD_DIM)
                            nc.vector.tensor_tensor(
                                out=at_tiles[pr][base:base + HEAD_DIM, :],
                                in0=pv[hh][0:HEAD_DIM, :], in1=rep_sb,
                                op=mybir.AluOpType.mult)

                        if pr == 1 and pending_wo is not None:
                            pending_wo()
                            pending_wo = None

                # ---- AllGather attn^T window across the 4 TP ranks ----
                if ag_mode == 'ppd':
                    gi = (rep * NW + w) % NGB
                    bar = None
                    if not no_ag:
                        bar = nc.gpsimd.collective_compute(
                            "AllGather", mybir.AluOpType.bypass,
                            replica_groups=groups,
                            ins=[bar_in[gi][:, :]],
                            outs=[bar_out[gi][:, :]],
                        )
                        for d in win_dmas:
                            bass._add_dep_helper(bar.ins, d.ins, sync=True,
                                                 reason="ppd barrier waits data")
                    win_dmas = []
                    pending_wo = (lambda w_=w, gi_=gi, bar_=bar:
                                  emit_wo_ppd(w_, gi_, bar_))
                elif ag_mode == 'pp':
                    pending_wo = (lambda w_=w,
                                  i0_=((rep * NW + w) * 4) % NCC:
                                  emit_wo_pp(w_, i0_))
                else:
                    pi = (rep * (NW // GSZ) + w // GSZ) % NCC
                    psl = slice((w % GSZ) * TCA, (w % GSZ + 1) * TCA)
                    for m in range(4):
                        nc.sync.dma_start(out=cc_in[pi][m * P:(m + 1) * P, psl],
                                          in_=at_tiles[m])
                    if w % GSZ == GSZ - 1:
                        if not no_ag:
                            nc.gpsimd.collective_compute(
                                "AllGather", mybir.AluOpType.bypass,
                                replica_groups=groups,
                                ins=[cc_in[pi][:, :]],
                                outs=[cc_out[pi][:, :]],
                            )
                        pending_wo = (lambda pi_=pi, w0_=w - GSZ + 1:
                                      emit_wo(pi_, w0_))

                if rep == 0 and w == 0:
                    for k in range(WOR // P):
                        nc.sync.dma_start(out=wo_sb[:, k, :], in_=wov[:, k, :])
                if not (w + 1 == NW and rep + 1 == repeat):
                    nw_ = (w + 1) % NW
                    nsl = slice(nw_ * TCA, (nw_ + 1) * TCA)
                    nslot = (w + 1) % 2
                    nc.sync.dma_start(out=cos_sb[:, nslot, :], in_=cosT[:, nsl])
                    nc.sync.dma_start(out=sin_sb[:, nslot, :], in_=sinTs[:, nsl])

        if pending_wo is not None:
            pending_wo()

    nc.compile()
    return nc


_NC_CACHE = {}

# Best measured configuration (updated as benchmarks land).
BUILD_OPTS = dict(ag_mode='full8', shared_cc=True, gsz=1, wo_mode='split',
                  interleave=True)


def _get_nc(T):
    key = (T, tuple(sorted(BUILD_OPTS.items())))
    if key not in _NC_CACHE:
        _NC_CACHE[key] = build_kernel(T, **BUILD_OPTS)
    return _NC_CACHE[key]


def _perm64():
    """Per-head permutation: interleaved (even,odd) -> [r(32) | i(32)]."""
    p = np.empty(HEAD_DIM, dtype=np.int64)
    p[:32] = np.arange(0, HEAD_DIM, 2)
    p[32:] = np.arange(1, HEAD_DIM, 2)
    return p


def make_inputs(x, freqs_cis, wq, wk, wv, wo, T, wo_mode='zeros', xbf=False,
                dmab=False):
    """Build the 8 per-core input maps (host-side sharding + layout prep)."""
    perm = _perm64()
    f32 = np.float32

    cos = np.asarray(freqs_cis[:T, :, 0], dtype=f32)   # [T, 32]
    sin = np.asarray(freqs_cis[:T, :, 1], dtype=f32)
    cosT = np.tile(cos.T, (4, 1)).astype(f32)                        # [128, T]
    sinTs = np.tile(np.vstack([-sin.T, sin.T]), (2, 1)).astype(f32)  # [128, T]

    J = np.zeros((HEAD_DIM, HEAD_DIM), dtype=f32)
    J[np.arange(32), np.arange(32) + 32] = 1.0
    J[np.arange(32) + 32, np.arange(32)] = 1.0
    swp = np.zeros((P, P), dtype=f32)
    swp[:HEAD_DIM, :HEAD_DIM] = J
    swp[HEAD_DIM:, HEAD_DIM:] = J

    # single causal triangle mask [128, 128]: msk[p, q] = (q >= p)
    q_idx = np.arange(P)
    p_idx = np.arange(P)[:, None]
    msk = (q_idx[None, :] >= p_idx).astype(f32)

    def permute_heads(w, n_heads):
        wh = np.asarray(w, f32).reshape(n_heads, HEAD_DIM, HIDDEN)
        return wh[:, perm, :].reshape(n_heads * HEAD_DIM, HIDDEN)

    wq_p = permute_heads(wq, N_HEADS)
    wk_p = permute_heads(wk, N_KV_HEADS)
    wv_n = np.asarray(wv, f32)
    wo_n = np.asarray(wo, f32)

    xdt = ml_dtypes.bfloat16 if xbf else f32
    in_maps = []
    for core in range(NCORES):
        b, j = divmod(core, NTP)
        if dmab:
            NCH = T // TCP
            xb = np.asarray(x[b, :T], f32)            # [T, H]
            xTc = np.ascontiguousarray(
                xb.reshape(NCH, TCP, HIDDEN // P, P).transpose(3, 0, 2, 1)
                .reshape(P, NCH * (HIDDEN // P) * TCP)).astype(xdt)
        else:
            xTc = np.ascontiguousarray(
                np.asarray(x[b, :T], f32).T).astype(xdt)
        wqTc = np.ascontiguousarray(wq_p[j * QF:(j + 1) * QF].T).astype(xdt)
        wkTc = np.ascontiguousarray(wk_p[j * KF:(j + 1) * KF].T).astype(xdt)
        wvTc = np.ascontiguousarray(wv_n[j * KF:(j + 1) * KF].T).astype(xdt)
        if wo_mode in ('split', 'pps'):
            # core c -> output cols [c*256, (c+1)*256) for BOTH batches
            wslice = wo_n[core * (COLS // 2):(core + 1) * (COLS // 2)]
            if wo_mode == 'pps':
                # row blocks ordered (pair pr, rank j, head e, dim)
                order = []
                for pr_ in range(4):
                    for j_ in range(4):
                        for e_ in range(2):
                            h_ = 8 * j_ + 2 * pr_ + e_
                            order.extend(range(h_ * HEAD_DIM,
                                               (h_ + 1) * HEAD_DIM))
                wslice = wslice[:, order]
            woTc = np.ascontiguousarray(wslice.T).astype(ml_dtypes.bfloat16)
        else:
            woTc = np.zeros((2 * HIDDEN, COLS), dtype=ml_dtypes.bfloat16)
            woTc[b * HIDDEN:(b + 1) * HIDDEN] = \
                wo_n[j * COLS:(j + 1) * COLS].T.astype(ml_dtypes.bfloat16)
        in_maps.append({
            "xT": xTc, "wqT": wqTc, "wkT": wkTc, "wvT": wvTc, "woT": woTc,
            "cosT": cosT, "sinTs": sinTs, "swp": swp, "msk": msk,
        })
    return in_maps


def assemble(outs, T, wo_mode='zeros'):
    """outs: list of 8 per-core "out" arrays [COLS, T] -> full [B, T, H]."""
    full = np.empty((B_FULL, T, HIDDEN), dtype=np.float32)
    HC = COLS // 2
    for core in range(NCORES):
        if wo_mode in ('split', 'pps'):
            csl = slice(core * HC, (core + 1) * HC)
            full[0, :, csl] = outs[core][0:HC].T
            full[1, :, csl] = outs[core][HC:COLS].T
        else:
            b, j = divmod(core, NTP)
            full[b, :, j * COLS:(j + 1) * COLS] = outs[core].T
    return full


def kernel(x, freqs_cis, wq, wk, wv, wo):
    global LAST_EXEC_NS, LAST_RESULTS
    T = x.shape[1]
    nc = _get_nc(T)
    wo_mode = BUILD_OPTS.get('wo_mode', 'zeros')
    in_maps = make_inputs(x, freqs_cis, wq, wk, wv, wo, T, wo_mode=wo_mode,
                          xbf=BUILD_OPTS.get('xbf', False),
                          dmab=BUILD_OPTS.get('dmab', False))
    trace = bool(int(os.environ.get("KERNEL_TRACE", "0")))
    res = run_bass_kernel_spmd(nc, in_maps, core_ids=list(range(NCORES)),
                               trace=trace)
    LAST_EXEC_NS = res.exec_time_ns
    LAST_RESULTS = res
    return assemble([res.results[c]["out"] for c in range(NCORES)], T, wo_mode)

